# revision 20
# baseline (speedup 1.0000x reference)
"""ResNet bottleneck block (dense_cnn) on 8 Trainium2 NeuronCores.

Reference computation (NCHW, fp32):
    t1  = relu(s1 * conv1x1(x, w1, stride=2) + b1)     # 512 -> 256, 28x28 -> 14x14
    t2  = relu(s2 * conv3x3(t1, w2, pad=1)   + b2)     # 256 -> 256
    t3  =      s3 * conv1x1(t2, w3)          + b3      # 256 -> 1024
    idn =      s4 * conv1x1(x, w4, stride=2) + b4      # 512 -> 1024
    out = relu(t3 + idn)                               # (64, 1024, 14, 14)

Strategy:
  - Data-parallel over batch: 64 images -> 8 cores x 8 images.
  - Host-side prep (numpy, cheap): subsample x to its even (h, w) positions
    (the only ones any conv reads), fold BN scales into conv weights,
    transpose weights to [ci, co] for the PE's stationary operand.
  - On-chip: every conv is a matmul with channels on partitions and
    (image, h, w) on the free dim.  The 3x3 conv is 9 shifted matmuls
    accumulating in PSUM over a zero-padded SBUF copy of t1 (16-wide rows,
    so each tap is a contiguous shifted view).
  - Residual branch accumulates into the same PSUM tile as conv3, so the
    add + final relu are free (one scalar-engine pass).
  - DMA: weights stream on the sync engine in consumption order; the input
    activations stream in parallel from the vector engine in per-group
    chunks so conv1 starts ~2us in; pad-buffer memsets go to gpsimd.

Activations use a compact 196-per-image layout except the 3x3 conv, whose
padded input planes are 16-wide (224 used columns of each 448-column
matmul; 2 junk columns per row are discarded by the psum->t2 copy).
"""

import os

import numpy as np

import concourse.mybir as mybir
import concourse.tile as tile
from concourse import bacc
from concourse.bass_utils import run_bass_kernel_spmd

F32 = mybir.dt.float32
BF16 = mybir.dt.bfloat16
F32R = mybir.dt.float32r
I32 = mybir.dt.int32

N_CORES = 8
B = 8              # images per core
HW = 14            # output spatial
P = HW * HW        # 196 per image plane (compact)
PB = B * P         # 1568
WP = 16            # padded row width for the 3x3 conv input
Q = HW * WP        # 224 (padded-plane columns per image in conv2 psum)
PADQ = 17 * WP     # 272 per-image padded plane (1 extra slack row)
NG = 2             # images per matmul group
G = B // NG        # 4 groups
NF = NG * P        # 392: compact moving-operand free size
NFQ = NG * Q       # 448: conv2 moving-operand free size

# Compute dtype for matmuls: "f32r" (fp32 storage, TF32-like multiply,
# full PE rate), "f32" (exact, 1/4 rate), "bf16".
COMPUTE_DT = os.environ.get("BOT_DT", "f32r")

_CACHE = {}


def _build_nc(reps=1):
    act_dt = {"bf16": BF16, "f32": F32, "f32r": F32R}[COMPUTE_DT]

    nc = bacc.Bacc()
    xs_d = nc.declare_dram_parameter("xs", [512, PB], act_dt, isOutput=False)
    w1_d = nc.declare_dram_parameter("w1t", [512, 256], act_dt, isOutput=False)
    w2_d = nc.declare_dram_parameter("w2t", [9 * 256, 256], act_dt, isOutput=False)
    w3_d = nc.declare_dram_parameter("w3t", [256, 1024], act_dt, isOutput=False)
    w4_d = nc.declare_dram_parameter("w4t", [512, 1024], act_dt, isOutput=False)
    b1_d = nc.declare_dram_parameter("b1p", [128, 2], F32, isOutput=False)
    b2_d = nc.declare_dram_parameter("b2p", [128, 2], F32, isOutput=False)
    b34_d = nc.declare_dram_parameter("b34p", [128, 8], F32, isOutput=False)
    out_d = nc.declare_dram_parameter("out", [1024, PB], F32, isOutput=True)

    relu = mybir.ActivationFunctionType.Relu
    alu_add = mybir.AluOpType.add
    alu_max = mybir.AluOpType.max

    def post(idx, dst, src, bias_ap):
        # relu(src + bias) -> dst, alternating between ACT and DVE so the
        # two engines share the psum-drain work
        if idx % 2 == 0:
            nc.scalar.activation(dst, src, relu, bias=bias_ap)
        else:
            nc.vector.tensor_scalar(dst, src, bias_ap, 0.0, alu_add, alu_max)

    with tile.TileContext(nc) as tc:
        with (
            tc.tile_pool(name="consts", bufs=1) as consts,
            tc.tile_pool(name="psum", bufs=8, space="PSUM") as psum,
            tc.tile_pool(name="outp", bufs=6) as outp,
        ):
            for _rep in range(reps):
                # --- weights on SP, in consumption order; the first
                # image-group's xs chunks go on SP too (HWDGE) so conv1
                # starts without paying the SWDGE first-byte latency ---
                xs_sb = [
                    consts.tile([128, PB], act_dt, tag=f"xs_{k}", name=f"xs_{k}")
                    for k in range(4)
                ]
                w1_sb = []
                for k in range(4):
                    t = consts.tile([128, 256], act_dt, tag=f"w1_{k}")
                    nc.sync.dma_start(out=t, in_=w1_d[k * 128:(k + 1) * 128, :])
                    w1_sb.append(t)
                    nc.sync.dma_start(
                        out=xs_sb[k][:, 0:NF], in_=xs_d[k * 128:(k + 1) * 128, 0:NF]
                    )
                b1_sb = consts.tile([128, 2], F32, tag="b1")
                nc.sync.dma_start(out=b1_sb, in_=b1_d[:, :])

                w2_sb = []
                for tap in range(9):
                    row = []
                    for k in range(2):
                        t = consts.tile([128, 256], act_dt, tag=f"w2_{tap}_{k}")
                        base = tap * 256 + k * 128
                        nc.sync.dma_start(out=t, in_=w2_d[base:base + 128, :])
                        row.append(t)
                    w2_sb.append(row)
                b2_sb = consts.tile([128, 2], F32, tag="b2")
                nc.sync.dma_start(out=b2_sb, in_=b2_d[:, :])

                w3_sb = []
                for k in range(2):
                    t = consts.tile([128, 1024], act_dt, tag=f"w3_{k}")
                    nc.sync.dma_start(out=t, in_=w3_d[k * 128:(k + 1) * 128, :])
                    w3_sb.append(t)
                w4_sb = []
                for k in range(4):
                    t = consts.tile([128, 1024], act_dt, tag=f"w4_{k}")
                    nc.sync.dma_start(out=t, in_=w4_d[k * 128:(k + 1) * 128, :])
                    w4_sb.append(t)
                b34_sb = consts.tile([128, 8], F32, tag="b34")
                nc.sync.dma_start(out=b34_sb, in_=b34_d[:, :])

                # --- remaining xs groups stream via gpsimd (SWDGE) ---
                for g in range(1, G):
                    for k in range(4):
                        nc.gpsimd.dma_start(
                            out=xs_sb[k][:, g * NF:(g + 1) * NF],
                            in_=xs_d[k * 128:(k + 1) * 128, g * NF:(g + 1) * NF],
                        )

                # --- zero-padded t1 planes (gpsimd) and t2 buffer ---
                t1pad = []
                for k in range(2):
                    t = consts.tile([128, B * PADQ], act_dt, tag=f"t1p_{k}")
                    if act_dt == F32R:
                        nc.vector.memset(t.bitcast(I32), 0)
                    else:
                        nc.vector.memset(t, 0.0)
                    t1pad.append(t)
                t2_sb = []
                for k in range(2):
                    t = consts.tile([128, PB], act_dt, tag=f"t2_{k}")
                    t2_sb.append(t)

                # --- stage 1: conv1 + relu, scattered into padded planes ---
                for g in range(G):
                    for m in range(2):
                        ps = psum.tile([128, NF], F32, tag="ps")
                        for k in range(4):
                            nc.tensor.matmul(
                                ps[:, :],
                                w1_sb[k][:, m * 128:(m + 1) * 128],
                                xs_sb[k][:, g * NF:(g + 1) * NF],
                                start=(k == 0),
                                stop=(k == 3),
                            )
                        for j in range(NG):
                            img = g * NG + j
                            src = ps[:, j * P:(j + 1) * P].rearrange(
                                "p (h w) -> p h w", w=HW
                            )
                            dst = t1pad[m][
                                :, img * PADQ:(img + 1) * PADQ
                            ].rearrange("p (h w) -> p h w", w=WP)[:, 1:15, 1:15]
                            post(g * 4 + m * 2 + j, dst, src,
                                 b1_sb[:, m:m + 1])

                # --- stage 2: conv2 (3x3 as 9 shifted matmuls) + relu ---
                for m in range(2):
                    for g in range(G):
                        ps = psum.tile([128, NFQ], F32, tag="ps")
                        i = 0
                        for tap in range(9):
                            dy, dx = divmod(tap, 3)
                            off = dy * WP + dx
                            for k in range(2):
                                seg = t1pad[k][
                                    :, g * NG * PADQ:(g * NG + NG) * PADQ
                                ].rearrange("p (n q) -> p n q", q=PADQ)[
                                    :, :, off:off + Q
                                ]
                                nc.tensor.matmul(
                                    ps[:, :],
                                    w2_sb[tap][k][:, m * 128:(m + 1) * 128],
                                    seg,
                                    start=(i == 0),
                                    stop=(i == 17),
                                )
                                i += 1
                        for j in range(NG):
                            img = g * NG + j
                            src = ps[:, j * Q:(j + 1) * Q].rearrange(
                                "p (h w) -> p h w", w=WP
                            )[:, :, 0:HW]
                            dst = t2_sb[m][:, img * P:(img + 1) * P].rearrange(
                                "p (h w) -> p h w", w=HW
                            )
                            post(m * 8 + g * 2 + j, dst, src,
                                 b2_sb[:, m:m + 1])

                # --- stage 3: conv3 + residual conv4 in one PSUM, relu ---
                for m in range(8):
                    for g in range(G):
                        ps = psum.tile([128, NF], F32, tag="ps")
                        for k in range(2):
                            nc.tensor.matmul(
                                ps[:, :],
                                w3_sb[k][:, m * 128:(m + 1) * 128],
                                t2_sb[k][:, g * NF:(g + 1) * NF],
                                start=(k == 0),
                                stop=False,
                            )
                        for k in range(4):
                            nc.tensor.matmul(
                                ps[:, :],
                                w4_sb[k][:, m * 128:(m + 1) * 128],
                                xs_sb[k][:, g * NF:(g + 1) * NF],
                                start=False,
                                stop=(k == 3),
                            )
                        ot = outp.tile([128, NF], F32, tag="ot")
                        post(m * 4 + g, ot, ps[:, :], b34_sb[:, m:m + 1])
                        nc.sync.dma_start(
                            out=out_d[m * 128:(m + 1) * 128, g * NF:(g + 1) * NF],
                            in_=ot,
                        )
    nc.finalize()
    return nc


def _prep(x, w1, w2, w3, w4, s1, b1, s2, b2, s3, b3, s4, b4):
    """Host-side input prep: shard, fold BN, transpose. All numpy."""
    if COMPUTE_DT == "bf16":
        import ml_dtypes

        cdt = np.dtype(ml_dtypes.bfloat16)
    else:
        cdt = np.dtype(np.float32)

    # x -> even positions, (core, c, n, h*14+w) channel-major partition lines
    xs = x[:, :, ::2, ::2].reshape(N_CORES, B, 512, P).transpose(0, 2, 1, 3)
    xs = np.ascontiguousarray(xs).reshape(N_CORES, 512, PB).astype(cdt)

    w1f = (w1[:, :, 0, 0] * s1[:, None]).T                    # (512, 256)
    w2f = w2 * s2[:, None, None, None]                        # (256,256,3,3)
    w2t = np.stack(
        [w2f[:, :, dy, dx].T for dy in range(3) for dx in range(3)]
    ).reshape(9 * 256, 256)                                   # (2304, 256)
    w3f = (w3[:, :, 0, 0] * s3[:, None]).T                    # (256, 1024)
    w4f = (w4[:, :, 0, 0] * s4[:, None]).T                    # (512, 1024)

    com = {
        "w1t": np.ascontiguousarray(w1f).astype(cdt),
        "w2t": np.ascontiguousarray(w2t).astype(cdt),
        "w3t": np.ascontiguousarray(w3f).astype(cdt),
        "w4t": np.ascontiguousarray(w4f).astype(cdt),
        "b1p": np.ascontiguousarray(b1.reshape(2, 128).T).astype(np.float32),
        "b2p": np.ascontiguousarray(b2.reshape(2, 128).T).astype(np.float32),
        "b34p": np.ascontiguousarray(
            (b3 + b4).reshape(8, 128).T
        ).astype(np.float32),
    }
    return [{"xs": xs[c], **com} for c in range(N_CORES)]


def _gather(results):
    out = np.empty((64, 1024, HW, HW), np.float32)
    for c, r in enumerate(results):
        o = r["out"].reshape(1024, B, HW, HW)
        out[c * B:(c + 1) * B] = o.transpose(1, 0, 2, 3)
    return out


def _get_nc(reps=1):
    key = ("nc", reps)
    if key not in _CACHE:
        _CACHE[key] = _build_nc(reps)
    return _CACHE[key]


def _run(in_maps, **kwargs):
    return run_bass_kernel_spmd(
        _get_nc(), in_maps, list(range(N_CORES)), **kwargs
    )


def kernel(**inputs):
    in_maps = _prep(**inputs)
    res = _run(in_maps)
    return _gather(res.results)


def _pjrt_runner(nc, in_maps):
    """Compile nc once; return (run_once, run_batch, results).

    run_once(): one blocking execution. run_batch(n): n pipelined
    executions, blocking at the end; returns elapsed seconds. results:
    first run's outputs as a list of per-core dicts.
    """
    import time

    import jax
    import numpy as np_
    from jax.sharding import Mesh, NamedSharding, PartitionSpec
    from jax.experimental.shard_map import shard_map

    from concourse import bass2jax, mybir as mb

    bass2jax.install_neuronx_cc_hook()
    part_name = nc.partition_id_tensor.name if nc.partition_id_tensor else None
    in_names, out_names, out_avals = [], [], []
    for alloc in nc.m.functions[0].allocations:
        if not isinstance(alloc, mb.MemoryLocationSet):
            continue
        name = alloc.memorylocations[0].name
        if alloc.kind == "ExternalInput":
            if name != part_name:
                in_names.append(name)
        elif alloc.kind == "ExternalOutput":
            out_names.append(name)
            out_avals.append(
                jax.core.ShapedArray(
                    tuple(alloc.tensor_shape), mb.dt.np(alloc.dtype)
                )
            )
    all_names = in_names + out_names + ([part_name] if part_name else [])

    def _body(*args):
        operands = list(args)
        if part_name is not None:
            operands.append(bass2jax.partition_id_tensor())
        outs = bass2jax._bass_exec_p.bind(
            *operands,
            out_avals=tuple(out_avals),
            in_names=tuple(all_names),
            out_names=tuple(out_names),
            lowering_input_output_aliases=(),
            sim_require_finite=False,
            sim_require_nnan=False,
            nc=nc,
        )
        return tuple(outs)

    devices = jax.devices()[:N_CORES]
    mesh = Mesh(np_.asarray(devices), ("core",))
    nspec = len(in_names) + len(out_names)
    sharded = jax.jit(
        shard_map(
            _body,
            mesh=mesh,
            in_specs=(PartitionSpec("core"),) * nspec,
            out_specs=(PartitionSpec("core"),) * len(out_names),
            check_rep=False,
        ),
        keep_unused=True,
    )

    sh = NamedSharding(mesh, PartitionSpec("core"))
    dev_args = [
        jax.device_put(
            np_.concatenate([in_maps[c][n] for c in range(N_CORES)], axis=0), sh
        )
        for n in in_names
    ] + [
        jax.device_put(
            np_.zeros((N_CORES * a.shape[0], *a.shape[1:]), a.dtype), sh
        )
        for a in out_avals
    ]

    outs = jax.block_until_ready(sharded(*dev_args))  # compile + warm

    results = [
        {
            n: np_.asarray(outs[i]).reshape(N_CORES, *out_avals[i].shape)[c]
            for i, n in enumerate(out_names)
        }
        for c in range(N_CORES)
    ]

    def run_once():
        jax.block_until_ready(sharded(*dev_args))

    def run_batch(n):
        t0 = time.monotonic()
        r = None
        for _ in range(n):
            r = sharded(*dev_args)
        jax.block_until_ready(r)
        return time.monotonic() - t0

    return run_once, run_batch, results


def kernel_timed(**inputs):
    """Run + estimate device exec time (ns).

    NTFF profiling is unavailable under this axon client; estimate device
    time by interleaved pairwise single-exec deltas between a 1-rep NEFF
    and an R-rep NEFF (kernel body repeated R times inside one NEFF): the
    per-rep delta cancels the multi-ms, drifting axon dispatch overhead.
    Returns (out, exec_time_ns).
    """
    import time

    import numpy as np_

    reps = int(os.environ.get("BOT_BENCH_REPS", "17"))
    npairs = int(os.environ.get("BOT_BENCH_PAIRS", "100"))
    in_maps = _prep(**inputs)

    once1, _, res = _pjrt_runner(_get_nc(1), in_maps)
    out = _gather(res)
    onceR, _, _ = _pjrt_runner(_get_nc(reps), in_maps)

    for _ in range(4):
        once1()
        onceR()
    deltas = []
    for _ in range(npairs):
        t0 = time.monotonic()
        once1()
        ta = time.monotonic() - t0
        t0 = time.monotonic()
        onceR()
        tb = time.monotonic() - t0
        deltas.append((tb - ta) * 1e9)
    per_rep = int(np_.median(deltas) / (reps - 1))
    print(f"[bench] pairwise per-rep over {npairs} pairs, R={reps}: "
          f"{per_rep} ns (delta med {np_.median(deltas):.0f} ns)")
    return out, per_rep


# revision 21
# speedup vs baseline: 1.0112x; 1.0112x over previous
"""ResNet bottleneck block (dense_cnn) on 8 Trainium2 NeuronCores.

Reference computation (NCHW, fp32):
    t1  = relu(s1 * conv1x1(x, w1, stride=2) + b1)     # 512 -> 256, 28x28 -> 14x14
    t2  = relu(s2 * conv3x3(t1, w2, pad=1)   + b2)     # 256 -> 256
    t3  =      s3 * conv1x1(t2, w3)          + b3      # 256 -> 1024
    idn =      s4 * conv1x1(x, w4, stride=2) + b4      # 512 -> 1024
    out = relu(t3 + idn)                               # (64, 1024, 14, 14)

Strategy:
  - Data-parallel over batch: 64 images -> 8 cores x 8 images.
  - Host-side prep (numpy, cheap): subsample x to its even (h, w) positions
    (the only ones any conv reads), fold BN scales into conv weights,
    transpose weights to [ci, co] for the PE's stationary operand.
  - On-chip: every conv is a matmul with channels on partitions and
    (image, h, w) on the free dim.  The 3x3 conv is 9 shifted matmuls
    accumulating in PSUM over a zero-padded SBUF copy of t1 (16-wide rows,
    so each tap is a contiguous shifted view).
  - Residual branch accumulates into the same PSUM tile as conv3, so the
    add + final relu are free (one scalar-engine pass).
  - DMA: weights and the first image-group of x stream on the sync engine
    (HWDGE) in consumption order; the remaining x groups stream in
    parallel via gpsimd (SWDGE); pad-buffer memsets go to the vector
    engine.  conv1 starts ~2us into the kernel.

Activations use a compact 196-per-image layout except the 3x3 conv, whose
padded input planes are 16-wide (224 used columns of each 448-column
matmul; 2 junk columns per row are discarded by the psum->t2 copy).
"""

import os

import numpy as np

import concourse.mybir as mybir
import concourse.tile as tile
from concourse import bacc
from concourse.bass_utils import run_bass_kernel_spmd

F32 = mybir.dt.float32
BF16 = mybir.dt.bfloat16
F32R = mybir.dt.float32r
I32 = mybir.dt.int32

N_CORES = 8
B = 8              # images per core
HW = 14            # output spatial
P = HW * HW        # 196 per image plane (compact)
PB = B * P         # 1568
WP = 16            # padded row width for the 3x3 conv input
Q = HW * WP        # 224 (padded-plane columns per image in conv2 psum)
PADQ = 17 * WP     # 272 per-image padded plane (1 extra slack row)
NG = 2             # images per matmul group
G = B // NG        # 4 groups
NF = NG * P        # 392: compact moving-operand free size
NFQ = NG * Q       # 448: conv2 moving-operand free size

# Compute dtype for matmuls: "f32r" (fp32 storage, TF32-like multiply,
# full PE rate), "f32" (exact, 1/4 rate), "bf16".
COMPUTE_DT = os.environ.get("BOT_DT", "f32r")

_CACHE = {}


def _build_nc(reps=1):
    act_dt = {"bf16": BF16, "f32": F32, "f32r": F32R}[COMPUTE_DT]

    nc = bacc.Bacc()
    xs_d = nc.declare_dram_parameter("xs", [512, PB], act_dt, isOutput=False)
    w1_d = nc.declare_dram_parameter("w1t", [512, 256], act_dt, isOutput=False)
    w2_d = nc.declare_dram_parameter("w2t", [9 * 256, 256], act_dt, isOutput=False)
    w3_d = nc.declare_dram_parameter("w3t", [256, 1024], act_dt, isOutput=False)
    w4_d = nc.declare_dram_parameter("w4t", [512, 1024], act_dt, isOutput=False)
    b1_d = nc.declare_dram_parameter("b1p", [128, 2], F32, isOutput=False)
    b2_d = nc.declare_dram_parameter("b2p", [128, 2], F32, isOutput=False)
    b34_d = nc.declare_dram_parameter("b34p", [128, 8], F32, isOutput=False)
    out_d = nc.declare_dram_parameter("out", [1024, PB], F32, isOutput=True)

    relu = mybir.ActivationFunctionType.Relu
    alu_add = mybir.AluOpType.add
    alu_max = mybir.AluOpType.max

    def post(idx, dst, src, bias_ap):
        # relu(src + bias) -> dst, alternating between ACT and DVE so the
        # two engines share the psum-drain work
        if idx % 2 == 0:
            nc.scalar.activation(dst, src, relu, bias=bias_ap)
        else:
            nc.vector.tensor_scalar(dst, src, bias_ap, 0.0, alu_add, alu_max)

    with tile.TileContext(nc) as tc:
        with (
            tc.tile_pool(name="consts", bufs=1) as consts,
            tc.tile_pool(name="psum", bufs=8, space="PSUM") as psum,
            tc.tile_pool(name="outp", bufs=6) as outp,
        ):
            for _rep in range(reps):
                # --- weights on SP, in consumption order; the first
                # image-group's xs chunks go on SP too (HWDGE) so conv1
                # starts without paying the SWDGE first-byte latency ---
                xs_sb = [
                    consts.tile([128, PB], act_dt, tag=f"xs_{k}", name=f"xs_{k}")
                    for k in range(4)
                ]
                w1_sb = []
                for k in range(4):
                    t = consts.tile([128, 256], act_dt, tag=f"w1_{k}")
                    nc.sync.dma_start(out=t, in_=w1_d[k * 128:(k + 1) * 128, :])
                    w1_sb.append(t)
                    nc.sync.dma_start(
                        out=xs_sb[k][:, 0:NF], in_=xs_d[k * 128:(k + 1) * 128, 0:NF]
                    )
                b1_sb = consts.tile([128, 2], F32, tag="b1")
                nc.sync.dma_start(out=b1_sb, in_=b1_d[:, :])

                w2_sb = []
                for tap in range(9):
                    row = []
                    for k in range(2):
                        t = consts.tile([128, 256], act_dt, tag=f"w2_{tap}_{k}")
                        base = tap * 256 + k * 128
                        nc.sync.dma_start(out=t, in_=w2_d[base:base + 128, :])
                        row.append(t)
                    w2_sb.append(row)
                b2_sb = consts.tile([128, 2], F32, tag="b2")
                nc.sync.dma_start(out=b2_sb, in_=b2_d[:, :])

                w3_sb = []
                for k in range(2):
                    t = consts.tile([128, 1024], act_dt, tag=f"w3_{k}")
                    nc.sync.dma_start(out=t, in_=w3_d[k * 128:(k + 1) * 128, :])
                    w3_sb.append(t)
                w4_sb = []
                for k in range(4):
                    t = consts.tile([128, 1024], act_dt, tag=f"w4_{k}")
                    nc.sync.dma_start(out=t, in_=w4_d[k * 128:(k + 1) * 128, :])
                    w4_sb.append(t)
                b34_sb = consts.tile([128, 8], F32, tag="b34")
                nc.sync.dma_start(out=b34_sb, in_=b34_d[:, :])

                # --- remaining xs groups stream via gpsimd (SWDGE) ---
                for g in range(1, G):
                    for k in range(4):
                        nc.gpsimd.dma_start(
                            out=xs_sb[k][:, g * NF:(g + 1) * NF],
                            in_=xs_d[k * 128:(k + 1) * 128, g * NF:(g + 1) * NF],
                        )

                # --- zero-padded t1 planes (gpsimd) and t2 buffer ---
                t1pad = []
                for k in range(2):
                    t = consts.tile([128, B * PADQ], act_dt, tag=f"t1p_{k}")
                    if act_dt == F32R:
                        nc.vector.memset(t.bitcast(I32), 0)
                    else:
                        nc.vector.memset(t, 0.0)
                    t1pad.append(t)
                t2_sb = []
                for k in range(2):
                    t = consts.tile([128, PB], act_dt, tag=f"t2_{k}")
                    t2_sb.append(t)

                # --- stage 1: conv1 + relu, scattered into padded planes ---
                for g in range(G):
                    for m in range(2):
                        ps = psum.tile([128, NF], F32, tag="ps")
                        for k in range(4):
                            nc.tensor.matmul(
                                ps[:, :],
                                w1_sb[k][:, m * 128:(m + 1) * 128],
                                xs_sb[k][:, g * NF:(g + 1) * NF],
                                start=(k == 0),
                                stop=(k == 3),
                            )
                        for j in range(NG):
                            img = g * NG + j
                            src = ps[:, j * P:(j + 1) * P].rearrange(
                                "p (h w) -> p h w", w=HW
                            )
                            dst = t1pad[m][
                                :, img * PADQ:(img + 1) * PADQ
                            ].rearrange("p (h w) -> p h w", w=WP)[:, 1:15, 1:15]
                            post(g * 4 + m * 2 + j, dst, src,
                                 b1_sb[:, m:m + 1])

                # --- stage 2: conv2 (3x3 as 9 shifted matmuls) + relu ---
                for m in range(2):
                    for g in range(G):
                        ps = psum.tile([128, NFQ], F32, tag="ps")
                        i = 0
                        for tap in range(9):
                            dy, dx = divmod(tap, 3)
                            off = dy * WP + dx
                            for k in range(2):
                                seg = t1pad[k][
                                    :, g * NG * PADQ:(g * NG + NG) * PADQ
                                ].rearrange("p (n q) -> p n q", q=PADQ)[
                                    :, :, off:off + Q
                                ]
                                nc.tensor.matmul(
                                    ps[:, :],
                                    w2_sb[tap][k][:, m * 128:(m + 1) * 128],
                                    seg,
                                    start=(i == 0),
                                    stop=(i == 17),
                                )
                                i += 1
                        for j in range(NG):
                            img = g * NG + j
                            src = ps[:, j * Q:(j + 1) * Q].rearrange(
                                "p (h w) -> p h w", w=WP
                            )[:, :, 0:HW]
                            dst = t2_sb[m][:, img * P:(img + 1) * P].rearrange(
                                "p (h w) -> p h w", w=HW
                            )
                            post(m * 8 + g * 2 + j, dst, src,
                                 b2_sb[:, m:m + 1])

                # --- stage 3: conv3 + residual conv4 in one PSUM, relu ---
                for m in range(8):
                    for g in range(G):
                        ps = psum.tile([128, NF], F32, tag="ps")
                        for k in range(2):
                            nc.tensor.matmul(
                                ps[:, :],
                                w3_sb[k][:, m * 128:(m + 1) * 128],
                                t2_sb[k][:, g * NF:(g + 1) * NF],
                                start=(k == 0),
                                stop=False,
                            )
                        for k in range(4):
                            nc.tensor.matmul(
                                ps[:, :],
                                w4_sb[k][:, m * 128:(m + 1) * 128],
                                xs_sb[k][:, g * NF:(g + 1) * NF],
                                start=False,
                                stop=(k == 3),
                            )
                        ot = outp.tile([128, NF], F32, tag="ot")
                        post(m * 4 + g, ot, ps[:, :], b34_sb[:, m:m + 1])
                        nc.sync.dma_start(
                            out=out_d[m * 128:(m + 1) * 128, g * NF:(g + 1) * NF],
                            in_=ot,
                        )
    nc.finalize()
    return nc


def _prep(x, w1, w2, w3, w4, s1, b1, s2, b2, s3, b3, s4, b4):
    """Host-side input prep: shard, fold BN, transpose. All numpy."""
    if COMPUTE_DT == "bf16":
        import ml_dtypes

        cdt = np.dtype(ml_dtypes.bfloat16)
    else:
        cdt = np.dtype(np.float32)

    # x -> even positions, (core, c, n, h*14+w) channel-major partition lines
    xs = x[:, :, ::2, ::2].reshape(N_CORES, B, 512, P).transpose(0, 2, 1, 3)
    xs = np.ascontiguousarray(xs).reshape(N_CORES, 512, PB).astype(cdt)

    w1f = (w1[:, :, 0, 0] * s1[:, None]).T                    # (512, 256)
    w2f = w2 * s2[:, None, None, None]                        # (256,256,3,3)
    w2t = np.stack(
        [w2f[:, :, dy, dx].T for dy in range(3) for dx in range(3)]
    ).reshape(9 * 256, 256)                                   # (2304, 256)
    w3f = (w3[:, :, 0, 0] * s3[:, None]).T                    # (256, 1024)
    w4f = (w4[:, :, 0, 0] * s4[:, None]).T                    # (512, 1024)

    com = {
        "w1t": np.ascontiguousarray(w1f).astype(cdt),
        "w2t": np.ascontiguousarray(w2t).astype(cdt),
        "w3t": np.ascontiguousarray(w3f).astype(cdt),
        "w4t": np.ascontiguousarray(w4f).astype(cdt),
        "b1p": np.ascontiguousarray(b1.reshape(2, 128).T).astype(np.float32),
        "b2p": np.ascontiguousarray(b2.reshape(2, 128).T).astype(np.float32),
        "b34p": np.ascontiguousarray(
            (b3 + b4).reshape(8, 128).T
        ).astype(np.float32),
    }
    return [{"xs": xs[c], **com} for c in range(N_CORES)]


def _gather(results):
    out = np.empty((64, 1024, HW, HW), np.float32)
    for c, r in enumerate(results):
        o = r["out"].reshape(1024, B, HW, HW)
        out[c * B:(c + 1) * B] = o.transpose(1, 0, 2, 3)
    return out


def _get_nc(reps=1):
    key = ("nc", reps)
    if key not in _CACHE:
        _CACHE[key] = _build_nc(reps)
    return _CACHE[key]


def _run(in_maps, **kwargs):
    return run_bass_kernel_spmd(
        _get_nc(), in_maps, list(range(N_CORES)), **kwargs
    )


def kernel(**inputs):
    in_maps = _prep(**inputs)
    res = _run(in_maps)
    return _gather(res.results)


def _pjrt_runner(nc, in_maps):
    """Compile nc once; return (run_once, run_batch, results).

    run_once(): one blocking execution. run_batch(n): n pipelined
    executions, blocking at the end; returns elapsed seconds. results:
    first run's outputs as a list of per-core dicts.
    """
    import time

    import jax
    import numpy as np_
    from jax.sharding import Mesh, NamedSharding, PartitionSpec
    from jax.experimental.shard_map import shard_map

    from concourse import bass2jax, mybir as mb

    bass2jax.install_neuronx_cc_hook()
    part_name = nc.partition_id_tensor.name if nc.partition_id_tensor else None
    in_names, out_names, out_avals = [], [], []
    for alloc in nc.m.functions[0].allocations:
        if not isinstance(alloc, mb.MemoryLocationSet):
            continue
        name = alloc.memorylocations[0].name
        if alloc.kind == "ExternalInput":
            if name != part_name:
                in_names.append(name)
        elif alloc.kind == "ExternalOutput":
            out_names.append(name)
            out_avals.append(
                jax.core.ShapedArray(
                    tuple(alloc.tensor_shape), mb.dt.np(alloc.dtype)
                )
            )
    all_names = in_names + out_names + ([part_name] if part_name else [])

    def _body(*args):
        operands = list(args)
        if part_name is not None:
            operands.append(bass2jax.partition_id_tensor())
        outs = bass2jax._bass_exec_p.bind(
            *operands,
            out_avals=tuple(out_avals),
            in_names=tuple(all_names),
            out_names=tuple(out_names),
            lowering_input_output_aliases=(),
            sim_require_finite=False,
            sim_require_nnan=False,
            nc=nc,
        )
        return tuple(outs)

    devices = jax.devices()[:N_CORES]
    mesh = Mesh(np_.asarray(devices), ("core",))
    nspec = len(in_names) + len(out_names)
    sharded = jax.jit(
        shard_map(
            _body,
            mesh=mesh,
            in_specs=(PartitionSpec("core"),) * nspec,
            out_specs=(PartitionSpec("core"),) * len(out_names),
            check_rep=False,
        ),
        keep_unused=True,
    )

    sh = NamedSharding(mesh, PartitionSpec("core"))
    dev_args = [
        jax.device_put(
            np_.concatenate([in_maps[c][n] for c in range(N_CORES)], axis=0), sh
        )
        for n in in_names
    ] + [
        jax.device_put(
            np_.zeros((N_CORES * a.shape[0], *a.shape[1:]), a.dtype), sh
        )
        for a in out_avals
    ]

    outs = jax.block_until_ready(sharded(*dev_args))  # compile + warm

    results = [
        {
            n: np_.asarray(outs[i]).reshape(N_CORES, *out_avals[i].shape)[c]
            for i, n in enumerate(out_names)
        }
        for c in range(N_CORES)
    ]

    def run_once():
        jax.block_until_ready(sharded(*dev_args))

    def run_batch(n):
        t0 = time.monotonic()
        r = None
        for _ in range(n):
            r = sharded(*dev_args)
        jax.block_until_ready(r)
        return time.monotonic() - t0

    return run_once, run_batch, results


def kernel_timed(**inputs):
    """Run + estimate device exec time (ns).

    NTFF profiling is unavailable under this axon client; estimate device
    time by interleaved pairwise single-exec deltas between a 1-rep NEFF
    and an R-rep NEFF (kernel body repeated R times inside one NEFF): the
    per-rep delta cancels the multi-ms, drifting axon dispatch overhead.
    Returns (out, exec_time_ns).
    """
    import time

    import numpy as np_

    reps = int(os.environ.get("BOT_BENCH_REPS", "17"))
    npairs = int(os.environ.get("BOT_BENCH_PAIRS", "100"))
    in_maps = _prep(**inputs)

    once1, _, res = _pjrt_runner(_get_nc(1), in_maps)
    out = _gather(res)
    onceR, _, _ = _pjrt_runner(_get_nc(reps), in_maps)

    for _ in range(4):
        once1()
        onceR()
    deltas = []
    for _ in range(npairs):
        t0 = time.monotonic()
        once1()
        ta = time.monotonic() - t0
        t0 = time.monotonic()
        onceR()
        tb = time.monotonic() - t0
        deltas.append((tb - ta) * 1e9)
    per_rep = int(np_.median(deltas) / (reps - 1))
    print(f"[bench] pairwise per-rep over {npairs} pairs, R={reps}: "
          f"{per_rep} ns (delta med {np_.median(deltas):.0f} ns)")
    return out, per_rep


# revision 22
# speedup vs baseline: 3.3514x; 3.3143x over previous
"""ResNet bottleneck block (dense_cnn) on 8 Trainium2 NeuronCores.

Reference computation (NCHW, fp32):
    t1  = relu(s1 * conv1x1(x, w1, stride=2) + b1)     # 512 -> 256, 28x28 -> 14x14
    t2  = relu(s2 * conv3x3(t1, w2, pad=1)   + b2)     # 256 -> 256
    t3  =      s3 * conv1x1(t2, w3)          + b3      # 256 -> 1024
    idn =      s4 * conv1x1(x, w4, stride=2) + b4      # 512 -> 1024
    out = relu(t3 + idn)                               # (64, 1024, 14, 14)

Strategy:
  - Data-parallel over batch: 64 images -> 8 cores x 8 images.
  - Host-side prep (numpy, cheap): subsample x to its even (h, w) positions
    (the only ones any conv reads), fold BN scales into conv weights,
    transpose weights to [ci, co] for the PE's stationary operand.
  - On-chip: every conv is a matmul with channels on partitions and
    (image, h, w) on the free dim.  The 3x3 conv is 9 shifted matmuls
    accumulating in PSUM over a zero-padded SBUF copy of t1 (16-wide rows,
    so each tap is a contiguous shifted view).
  - Residual branch accumulates into the same PSUM tile as conv3, so the
    add + final relu are free (one scalar-engine pass).
  - DMA: weights and the first image-group of x stream on the sync engine
    (HWDGE) in consumption order; the remaining x groups stream in
    parallel via gpsimd (SWDGE); pad-buffer memsets go to the vector
    engine.  conv1 starts ~2us into the kernel.

Activations use a compact 196-per-image layout except the 3x3 conv, whose
padded input planes are 16-wide (224 used columns of each 448-column
matmul; 2 junk columns per row are discarded by the psum->t2 copy).
"""

import os

import numpy as np

import concourse.mybir as mybir
import concourse.tile as tile
from concourse import bacc
from concourse.bass_utils import run_bass_kernel_spmd

F32 = mybir.dt.float32
BF16 = mybir.dt.bfloat16
F32R = mybir.dt.float32r
I32 = mybir.dt.int32

N_CORES = 8
B = 8              # images per core
HW = 14            # output spatial
P = HW * HW        # 196 per image plane (compact)
PB = B * P         # 1568
WP = 16            # padded row width for the 3x3 conv input
Q = HW * WP        # 224 (padded-plane columns per image in conv2 psum)
PADQ = 17 * WP     # 272 per-image padded plane (1 extra slack row)
NG = 2             # images per matmul group
G = B // NG        # 4 groups
NF = NG * P        # 392: compact moving-operand free size
NFQ = NG * Q       # 448: conv2 moving-operand free size

# Compute dtype for matmuls: "f32r" (fp32 storage, TF32-like multiply,
# full PE rate), "f32" (exact, 1/4 rate), "bf16".
COMPUTE_DT = os.environ.get("BOT_DT", "f32r")

_CACHE = {}


def _build_nc(reps=1):
    act_dt = {"bf16": BF16, "f32": F32, "f32r": F32R}[COMPUTE_DT]

    nc = bacc.Bacc()
    xs_d = nc.declare_dram_parameter("xs", [512, PB], act_dt, isOutput=False)
    w1_d = nc.declare_dram_parameter("w1t", [512, 256], act_dt, isOutput=False)
    w2_d = nc.declare_dram_parameter("w2t", [9 * 256, 256], act_dt, isOutput=False)
    w3_d = nc.declare_dram_parameter("w3t", [256, 1024], act_dt, isOutput=False)
    w4_d = nc.declare_dram_parameter("w4t", [512, 1024], act_dt, isOutput=False)
    b1_d = nc.declare_dram_parameter("b1p", [128, 2], F32, isOutput=False)
    b2_d = nc.declare_dram_parameter("b2p", [128, 2], F32, isOutput=False)
    b34_d = nc.declare_dram_parameter("b34p", [128, 8], F32, isOutput=False)
    out_d = nc.declare_dram_parameter("out", [1024, PB], F32, isOutput=True)

    relu = mybir.ActivationFunctionType.Relu
    alu_add = mybir.AluOpType.add
    alu_max = mybir.AluOpType.max

    def post(idx, dst, src, bias_ap):
        # relu(src + bias) -> dst, alternating between ACT and DVE so the
        # two engines share the psum-drain work
        if idx % 2 == 0:
            nc.scalar.activation(dst, src, relu, bias=bias_ap)
        else:
            nc.vector.tensor_scalar(dst, src, bias_ap, 0.0, alu_add, alu_max)

    with tile.TileContext(nc) as tc:
        with (
            tc.tile_pool(name="consts", bufs=1) as consts,
            tc.tile_pool(name="psum", bufs=8, space="PSUM") as psum,
            tc.tile_pool(name="outp", bufs=6) as outp,
        ):
            for _rep in range(reps):
                # --- weights on SP, in consumption order; the first
                # image-group's xs chunks go on SP too (HWDGE) so conv1
                # starts without paying the SWDGE first-byte latency ---
                xs_sb = [
                    consts.tile([128, PB], act_dt, tag=f"xs_{k}", name=f"xs_{k}")
                    for k in range(4)
                ]
                w1_sb = []
                for k in range(4):
                    t = consts.tile([128, 256], act_dt, tag=f"w1_{k}")
                    nc.sync.dma_start(out=t, in_=w1_d[k * 128:(k + 1) * 128, :])
                    w1_sb.append(t)
                    nc.sync.dma_start(
                        out=xs_sb[k][:, 0:NF], in_=xs_d[k * 128:(k + 1) * 128, 0:NF]
                    )
                b1_sb = consts.tile([128, 2], F32, tag="b1")
                nc.sync.dma_start(out=b1_sb, in_=b1_d[:, :])

                w2_sb = []
                for tap in range(9):
                    row = []
                    for k in range(2):
                        t = consts.tile([128, 256], act_dt, tag=f"w2_{tap}_{k}")
                        base = tap * 256 + k * 128
                        nc.sync.dma_start(out=t, in_=w2_d[base:base + 128, :])
                        row.append(t)
                    w2_sb.append(row)
                b2_sb = consts.tile([128, 2], F32, tag="b2")
                nc.sync.dma_start(out=b2_sb, in_=b2_d[:, :])

                w3_sb = []
                for k in range(2):
                    t = consts.tile([128, 1024], act_dt, tag=f"w3_{k}")
                    nc.sync.dma_start(out=t, in_=w3_d[k * 128:(k + 1) * 128, :])
                    w3_sb.append(t)
                w4_sb = []
                for k in range(4):
                    t = consts.tile([128, 1024], act_dt, tag=f"w4_{k}")
                    nc.sync.dma_start(out=t, in_=w4_d[k * 128:(k + 1) * 128, :])
                    w4_sb.append(t)
                b34_sb = consts.tile([128, 8], F32, tag="b34")
                nc.sync.dma_start(out=b34_sb, in_=b34_d[:, :])

                # --- remaining xs groups stream via gpsimd (SWDGE) ---
                for g in range(1, G):
                    for k in range(4):
                        nc.gpsimd.dma_start(
                            out=xs_sb[k][:, g * NF:(g + 1) * NF],
                            in_=xs_d[k * 128:(k + 1) * 128, g * NF:(g + 1) * NF],
                        )

                # --- zero-padded t1 planes (gpsimd) and t2 buffer ---
                t1pad = []
                for k in range(2):
                    t = consts.tile([128, B * PADQ], act_dt, tag=f"t1p_{k}")
                    if act_dt == F32R:
                        nc.vector.memset(t.bitcast(I32), 0)
                    else:
                        nc.vector.memset(t, 0.0)
                    t1pad.append(t)
                t2_sb = []
                for k in range(2):
                    t = consts.tile([128, PB], act_dt, tag=f"t2_{k}")
                    t2_sb.append(t)

                # --- stage 1: conv1 + relu, scattered into padded planes ---
                for g in range(G):
                    for m in range(2):
                        ps = psum.tile([128, NF], F32, tag="ps")
                        for k in range(4):
                            nc.tensor.matmul(
                                ps[:, :],
                                w1_sb[k][:, m * 128:(m + 1) * 128],
                                xs_sb[k][:, g * NF:(g + 1) * NF],
                                start=(k == 0),
                                stop=(k == 3),
                            )
                        for j in range(NG):
                            img = g * NG + j
                            src = ps[:, j * P:(j + 1) * P].rearrange(
                                "p (h w) -> p h w", w=HW
                            )
                            dst = t1pad[m][
                                :, img * PADQ:(img + 1) * PADQ
                            ].rearrange("p (h w) -> p h w", w=WP)[:, 1:15, 1:15]
                            post(g * 4 + m * 2 + j, dst, src,
                                 b1_sb[:, m:m + 1])

                # --- stage 2: conv2 (3x3 as 9 shifted matmuls) + relu ---
                # moving operand uses a 3-level AP [image, h(step 16), w] so
                # the 2 pad columns per 16-wide row are never streamed
                for m in range(2):
                    for g in range(G):
                        ps = psum.tile([128, NF], F32, tag="ps")
                        i = 0
                        for tap in range(9):
                            dy, dx = divmod(tap, 3)
                            for k in range(2):
                                seg = t1pad[k][
                                    :, g * NG * PADQ:(g * NG + NG) * PADQ
                                ].rearrange(
                                    "p (n h w) -> p n h w", h=17, w=WP
                                )[:, :, dy:dy + HW, dx:dx + HW]
                                nc.tensor.matmul(
                                    ps[:, :],
                                    w2_sb[tap][k][:, m * 128:(m + 1) * 128],
                                    seg,
                                    start=(i == 0),
                                    stop=(i == 17),
                                )
                                i += 1
                        for j in range(NG):
                            img = g * NG + j
                            dst = t2_sb[m][:, img * P:(img + 1) * P]
                            post(m * 8 + g * 2 + j, dst,
                                 ps[:, j * P:(j + 1) * P],
                                 b2_sb[:, m:m + 1])

                # --- stage 3: conv3 + residual conv4 in one PSUM, relu ---
                for m in range(8):
                    for g in range(G):
                        ps = psum.tile([128, NF], F32, tag="ps")
                        for k in range(2):
                            nc.tensor.matmul(
                                ps[:, :],
                                w3_sb[k][:, m * 128:(m + 1) * 128],
                                t2_sb[k][:, g * NF:(g + 1) * NF],
                                start=(k == 0),
                                stop=False,
                            )
                        for k in range(4):
                            nc.tensor.matmul(
                                ps[:, :],
                                w4_sb[k][:, m * 128:(m + 1) * 128],
                                xs_sb[k][:, g * NF:(g + 1) * NF],
                                start=False,
                                stop=(k == 3),
                            )
                        ot = outp.tile([128, NF], F32, tag="ot")
                        post(m * 4 + g, ot, ps[:, :], b34_sb[:, m:m + 1])
                        nc.sync.dma_start(
                            out=out_d[m * 128:(m + 1) * 128, g * NF:(g + 1) * NF],
                            in_=ot,
                        )
    nc.finalize()
    return nc


def _prep(x, w1, w2, w3, w4, s1, b1, s2, b2, s3, b3, s4, b4):
    """Host-side input prep: shard, fold BN, transpose. All numpy."""
    if COMPUTE_DT == "bf16":
        import ml_dtypes

        cdt = np.dtype(ml_dtypes.bfloat16)
    else:
        cdt = np.dtype(np.float32)

    # x -> even positions, (core, c, n, h*14+w) channel-major partition lines
    xs = x[:, :, ::2, ::2].reshape(N_CORES, B, 512, P).transpose(0, 2, 1, 3)
    xs = np.ascontiguousarray(xs).reshape(N_CORES, 512, PB).astype(cdt)

    w1f = (w1[:, :, 0, 0] * s1[:, None]).T                    # (512, 256)
    w2f = w2 * s2[:, None, None, None]                        # (256,256,3,3)
    w2t = np.stack(
        [w2f[:, :, dy, dx].T for dy in range(3) for dx in range(3)]
    ).reshape(9 * 256, 256)                                   # (2304, 256)
    w3f = (w3[:, :, 0, 0] * s3[:, None]).T                    # (256, 1024)
    w4f = (w4[:, :, 0, 0] * s4[:, None]).T                    # (512, 1024)

    com = {
        "w1t": np.ascontiguousarray(w1f).astype(cdt),
        "w2t": np.ascontiguousarray(w2t).astype(cdt),
        "w3t": np.ascontiguousarray(w3f).astype(cdt),
        "w4t": np.ascontiguousarray(w4f).astype(cdt),
        "b1p": np.ascontiguousarray(b1.reshape(2, 128).T).astype(np.float32),
        "b2p": np.ascontiguousarray(b2.reshape(2, 128).T).astype(np.float32),
        "b34p": np.ascontiguousarray(
            (b3 + b4).reshape(8, 128).T
        ).astype(np.float32),
    }
    return [{"xs": xs[c], **com} for c in range(N_CORES)]


def _gather(results):
    out = np.empty((64, 1024, HW, HW), np.float32)
    for c, r in enumerate(results):
        o = r["out"].reshape(1024, B, HW, HW)
        out[c * B:(c + 1) * B] = o.transpose(1, 0, 2, 3)
    return out


def _get_nc(reps=1):
    key = ("nc", reps)
    if key not in _CACHE:
        _CACHE[key] = _build_nc(reps)
    return _CACHE[key]


def _run(in_maps, **kwargs):
    return run_bass_kernel_spmd(
        _get_nc(), in_maps, list(range(N_CORES)), **kwargs
    )


def kernel(**inputs):
    in_maps = _prep(**inputs)
    res = _run(in_maps)
    return _gather(res.results)


def _pjrt_runner(nc, in_maps):
    """Compile nc once; return (run_once, run_batch, results).

    run_once(): one blocking execution. run_batch(n): n pipelined
    executions, blocking at the end; returns elapsed seconds. results:
    first run's outputs as a list of per-core dicts.
    """
    import time

    import jax
    import numpy as np_
    from jax.sharding import Mesh, NamedSharding, PartitionSpec
    from jax.experimental.shard_map import shard_map

    from concourse import bass2jax, mybir as mb

    bass2jax.install_neuronx_cc_hook()
    part_name = nc.partition_id_tensor.name if nc.partition_id_tensor else None
    in_names, out_names, out_avals = [], [], []
    for alloc in nc.m.functions[0].allocations:
        if not isinstance(alloc, mb.MemoryLocationSet):
            continue
        name = alloc.memorylocations[0].name
        if alloc.kind == "ExternalInput":
            if name != part_name:
                in_names.append(name)
        elif alloc.kind == "ExternalOutput":
            out_names.append(name)
            out_avals.append(
                jax.core.ShapedArray(
                    tuple(alloc.tensor_shape), mb.dt.np(alloc.dtype)
                )
            )
    all_names = in_names + out_names + ([part_name] if part_name else [])

    def _body(*args):
        operands = list(args)
        if part_name is not None:
            operands.append(bass2jax.partition_id_tensor())
        outs = bass2jax._bass_exec_p.bind(
            *operands,
            out_avals=tuple(out_avals),
            in_names=tuple(all_names),
            out_names=tuple(out_names),
            lowering_input_output_aliases=(),
            sim_require_finite=False,
            sim_require_nnan=False,
            nc=nc,
        )
        return tuple(outs)

    devices = jax.devices()[:N_CORES]
    mesh = Mesh(np_.asarray(devices), ("core",))
    nspec = len(in_names) + len(out_names)
    sharded = jax.jit(
        shard_map(
            _body,
            mesh=mesh,
            in_specs=(PartitionSpec("core"),) * nspec,
            out_specs=(PartitionSpec("core"),) * len(out_names),
            check_rep=False,
        ),
        keep_unused=True,
    )

    sh = NamedSharding(mesh, PartitionSpec("core"))
    dev_args = [
        jax.device_put(
            np_.concatenate([in_maps[c][n] for c in range(N_CORES)], axis=0), sh
        )
        for n in in_names
    ] + [
        jax.device_put(
            np_.zeros((N_CORES * a.shape[0], *a.shape[1:]), a.dtype), sh
        )
        for a in out_avals
    ]

    outs = jax.block_until_ready(sharded(*dev_args))  # compile + warm

    results = [
        {
            n: np_.asarray(outs[i]).reshape(N_CORES, *out_avals[i].shape)[c]
            for i, n in enumerate(out_names)
        }
        for c in range(N_CORES)
    ]

    def run_once():
        jax.block_until_ready(sharded(*dev_args))

    def run_batch(n):
        t0 = time.monotonic()
        r = None
        for _ in range(n):
            r = sharded(*dev_args)
        jax.block_until_ready(r)
        return time.monotonic() - t0

    return run_once, run_batch, results


def kernel_timed(**inputs):
    """Run + estimate device exec time (ns).

    NTFF profiling is unavailable under this axon client; estimate device
    time by interleaved pairwise single-exec deltas between a 1-rep NEFF
    and an R-rep NEFF (kernel body repeated R times inside one NEFF): the
    per-rep delta cancels the multi-ms, drifting axon dispatch overhead.
    Returns (out, exec_time_ns).
    """
    import time

    import numpy as np_

    reps = int(os.environ.get("BOT_BENCH_REPS", "17"))
    npairs = int(os.environ.get("BOT_BENCH_PAIRS", "100"))
    in_maps = _prep(**inputs)

    once1, _, res = _pjrt_runner(_get_nc(1), in_maps)
    out = _gather(res)
    onceR, _, _ = _pjrt_runner(_get_nc(reps), in_maps)

    for _ in range(4):
        once1()
        onceR()
    deltas = []
    for _ in range(npairs):
        t0 = time.monotonic()
        once1()
        ta = time.monotonic() - t0
        t0 = time.monotonic()
        onceR()
        tb = time.monotonic() - t0
        deltas.append((tb - ta) * 1e9)
    per_rep = int(np_.median(deltas) / (reps - 1))
    print(f"[bench] pairwise per-rep over {npairs} pairs, R={reps}: "
          f"{per_rep} ns (delta med {np_.median(deltas):.0f} ns)")
    return out, per_rep
